# revision 10
# baseline (speedup 1.0000x reference)
"""Single-head attention (B=4, S=2048, D=1024) on 8 TRN2 NeuronCores.

Sharding: each core owns one (batch, seq-half) block of 1024 queries.
The two cores of a batch each recompute K/V for the full 2048-row
sequence (no collectives needed; ~29% duplicated projection FLOPs).

Layout trick: all tensors are kept feature-major ("transposed") on chip.
 - host sends xT = x[b].T as [D, S] (with the core's own query half
   rotated to the front so a compile-time slice [:, :1024] is SPMD-safe)
 - projections produce QT/KT = [e, s] (stationary W tiles, moving xT)
   and V = [s, e] natural (stationary xT tiles, moving Wv)
 - scoresT[k, q] = KT.T-contracted matmul; softmax runs along the
   partition (k) axis: exp on ScalarE, column sums via ones-vector
   matmul, reciprocal on VectorE, broadcast via fp32 outer-product
   matmul
 - out^T[e, q] = V.T @ P^T accumulated over k tiles; normalized by the
   broadcast reciprocal on the PSUM->SBUF copy; host transposes back.
"""

import os

import numpy as np
import ml_dtypes

import concourse.bass as bass
from concourse import bacc
import concourse.mybir as mybir
import concourse.tile as tile
from concourse.bass_utils import run_bass_kernel_spmd

BF = mybir.dt.bfloat16
F32 = mybir.dt.float32

B, S, D = 4, 2048, 1024
SQ = S // 2          # queries per core
NDT = D // 128       # 8 d-tiles (contraction of projections)
NET = D // 128       # 8 e-tiles (feature dim of q/k/v)
NKT = S // 128       # 16 k-tiles (keys)
NQC = SQ // 512      # 2 query chunks of 512
NSC = S // 512       # 4 sequence chunks of 512
NEC = D // 512       # 2 feature chunks of 512

# exec_time_ns of the last traced run (None when BASS_TRACE is unset)
LAST_EXEC_TIME_NS = None
LAST_TRACE = None


def _build():
    nc = bacc.Bacc(None)

    xt_ext = nc.declare_dram_parameter("xt", [D, S], BF, isOutput=False)
    wq_ext = nc.declare_dram_parameter("wq", [D, D], BF, isOutput=False)
    wk_ext = nc.declare_dram_parameter("wk", [D, D], BF, isOutput=False)
    wv_ext = nc.declare_dram_parameter("wv", [D, D], BF, isOutput=False)
    bq_ext = nc.declare_dram_parameter("bq", [D], F32, isOutput=False)
    bk_ext = nc.declare_dram_parameter("bk", [D], F32, isOutput=False)
    bv_ext = nc.declare_dram_parameter("bv", [D], BF, isOutput=False)
    out_ext = nc.declare_dram_parameter("out", [D, SQ], F32, isOutput=True)

    with tile.TileContext(nc) as tc:
        with (
            tc.tile_pool(name="xt", bufs=NDT) as xt_pool,
            tc.tile_pool(name="wst", bufs=3) as wst_pool,
            tc.tile_pool(name="wv", bufs=NDT) as wv_pool,
            tc.tile_pool(name="qt", bufs=NET) as qt_pool,
            tc.tile_pool(name="kt", bufs=NET) as kt_pool,
            tc.tile_pool(name="vv", bufs=NKT) as v_pool,
            tc.tile_pool(name="pt", bufs=NKT) as pt_pool,
            tc.tile_pool(name="small", bufs=1) as small,
            tc.tile_pool(name="ot", bufs=4) as ot_pool,
            tc.tile_pool(name="ps", bufs=6, space="PSUM") as ps_pool,
            tc.tile_pool(name="pcs", bufs=2, space="PSUM") as pcs_pool,
        ):
            # ---- input loads ----
            xt = []
            for dt in range(NDT):
                t = xt_pool.tile([128, S], BF, tag="xt")
                nc.sync.dma_start(out=t, in_=xt_ext[dt * 128 : (dt + 1) * 128, :])
                xt.append(t)
            wv_sb = []
            for dt in range(NDT):
                t = wv_pool.tile([128, D], BF, tag="wv")
                nc.sync.dma_start(out=t, in_=wv_ext[dt * 128 : (dt + 1) * 128, :])
                wv_sb.append(t)
            bq_sb = small.tile([128, NET], F32, tag="bq")
            nc.sync.dma_start(out=bq_sb, in_=bq_ext[:].rearrange("(e p) -> p e", p=128))
            bk_sb = small.tile([128, NET], F32, tag="bk")
            nc.sync.dma_start(out=bk_sb, in_=bk_ext[:].rearrange("(e p) -> p e", p=128))
            bv_sb = small.tile([1, D], BF, tag="bv")
            nc.sync.dma_start(out=bv_sb, in_=bv_ext[:].rearrange("(o d) -> o d", o=1))
            ones_col = small.tile([128, 1], BF, tag="onc")
            nc.vector.memset(ones_col, 1.0)
            ones_row = small.tile([1, 128], BF, tag="onr")
            nc.vector.memset(ones_row, 1.0)
            ones_row_f = small.tile([1, 128], F32, tag="onrf")
            nc.vector.memset(ones_row_f, 1.0)

            wq_r = wq_ext[:].rearrange("(t p) e -> p t e", p=128)
            wk_r = wk_ext[:].rearrange("(t p) e -> p t e", p=128)

            # ---- QT[e,s] = (Wq.T @ xT) + bq, queries only (first SQ cols) ----
            qt_sb = []
            for et in range(NET):
                w_t = wst_pool.tile([128, NDT, 128], BF, tag="wst")
                nc.sync.dma_start(out=w_t, in_=wq_r[:, :, et * 128 : (et + 1) * 128])
                acc = [ps_pool.tile([128, 512], F32, tag="acc", name="acc") for _ in range(NQC)]
                for dt in range(NDT):
                    for q in range(NQC):
                        nc.tensor.matmul(
                            acc[q],
                            lhsT=w_t[:, dt, :],
                            rhs=xt[dt][:, q * 512 : (q + 1) * 512],
                            start=(dt == 0),
                            stop=(dt == NDT - 1),
                        )
                qt_t = qt_pool.tile([128, SQ], BF, tag="qt")
                for q in range(NQC):
                    nc.scalar.activation(
                        out=qt_t[:, q * 512 : (q + 1) * 512],
                        in_=acc[q],
                        func=mybir.ActivationFunctionType.Identity,
                        bias=bq_sb[:, et : et + 1],
                    )
                qt_sb.append(qt_t)

            # ---- KT[e,s] = (Wk.T @ xT) + bk, full sequence ----
            kt_sb = []
            for et in range(NET):
                w_t = wst_pool.tile([128, NDT, 128], BF, tag="wst")
                nc.sync.dma_start(out=w_t, in_=wk_r[:, :, et * 128 : (et + 1) * 128])
                acc = [ps_pool.tile([128, 512], F32, tag="acc", name="acc") for _ in range(NSC)]
                for dt in range(NDT):
                    for s in range(NSC):
                        nc.tensor.matmul(
                            acc[s],
                            lhsT=w_t[:, dt, :],
                            rhs=xt[dt][:, s * 512 : (s + 1) * 512],
                            start=(dt == 0),
                            stop=(dt == NDT - 1),
                        )
                kt_t = kt_pool.tile([128, S], BF, tag="kt")
                for s in range(NSC):
                    nc.scalar.activation(
                        out=kt_t[:, s * 512 : (s + 1) * 512],
                        in_=acc[s],
                        func=mybir.ActivationFunctionType.Identity,
                        bias=bk_sb[:, et : et + 1],
                    )
                kt_sb.append(kt_t)

            # ---- V[s,e] = (xT.T @ Wv) + bv, natural layout, full sequence ----
            v_sb = []
            for st in range(NKT):
                acc = [ps_pool.tile([128, 512], F32, tag="acc", name="acc") for _ in range(NEC)]
                for dt in range(NDT):
                    for e in range(NEC):
                        nc.tensor.matmul(
                            acc[e],
                            lhsT=xt[dt][:, st * 128 : (st + 1) * 128],
                            rhs=wv_sb[dt][:, e * 512 : (e + 1) * 512],
                            start=(dt == 0),
                            stop=False,
                        )
                for e in range(NEC):
                    # bias: ones[1,128].T @ bv_row[1,512] broadcasts bv
                    nc.tensor.matmul(
                        acc[e],
                        lhsT=ones_row,
                        rhs=bv_sb[:, e * 512 : (e + 1) * 512],
                        start=False,
                        stop=True,
                    )
                v_t = v_pool.tile([128, D], BF, tag="vv")
                for e in range(NEC):
                    nc.vector.tensor_copy(
                        out=v_t[:, e * 512 : (e + 1) * 512], in_=acc[e]
                    )
                v_sb.append(v_t)

            # ---- scoresT[k,q] = KT.T-contraction; P = exp(scoresT) ----
            # column sums accumulate in dedicated PSUM banks across all k
            cs_acc = [pcs_pool.tile([1, 512], F32, tag="cs", name="cs") for _ in range(NQC)]
            pt_sb = []
            for kt in range(NKT):
                acc = [ps_pool.tile([128, 512], F32, tag="acc", name="acc") for _ in range(NQC)]
                for et in range(NET):
                    for q in range(NQC):
                        nc.tensor.matmul(
                            acc[q],
                            lhsT=kt_sb[et][:, kt * 128 : (kt + 1) * 128],
                            rhs=qt_sb[et][:, q * 512 : (q + 1) * 512],
                            start=(et == 0),
                            stop=(et == NET - 1),
                        )
                pt_t = pt_pool.tile([128, SQ], BF, tag="pt")
                for q in range(NQC):
                    nc.scalar.activation(
                        out=pt_t[:, q * 512 : (q + 1) * 512],
                        in_=acc[q],
                        func=mybir.ActivationFunctionType.Exp,
                    )
                for q in range(NQC):
                    nc.tensor.matmul(
                        cs_acc[q],
                        lhsT=ones_col,
                        rhs=pt_t[:, q * 512 : (q + 1) * 512],
                        start=(kt == 0),
                        stop=(kt == NKT - 1),
                    )
                pt_sb.append(pt_t)

            # ---- softmax denominator: reciprocal + broadcast to 128 parts ----
            cs_sb = small.tile([1, SQ], F32, tag="css")
            for q in range(NQC):
                nc.vector.tensor_copy(
                    out=cs_sb[:, q * 512 : (q + 1) * 512], in_=cs_acc[q]
                )
            rc_sb = small.tile([1, SQ], F32, tag="rcs")
            nc.vector.reciprocal(out=rc_sb, in_=cs_sb)
            bc_sb = small.tile([128, SQ], F32, tag="bcs")
            for q in range(NQC):
                bc_ps = pcs_pool.tile([128, 512], F32, tag="cs", name="bc")
                nc.tensor.matmul(
                    bc_ps,
                    lhsT=ones_row_f,
                    rhs=rc_sb[:, q * 512 : (q + 1) * 512],
                    start=True,
                    stop=True,
                )
                nc.vector.tensor_copy(
                    out=bc_sb[:, q * 512 : (q + 1) * 512], in_=bc_ps
                )

            # ---- outT[e,q] = V.T @ P^T, normalized on the way out ----
            for et in range(NET):
                acc = [ps_pool.tile([128, 512], F32, tag="acc", name="acc") for _ in range(NQC)]
                for kt in range(NKT):
                    for q in range(NQC):
                        nc.tensor.matmul(
                            acc[q],
                            lhsT=v_sb[kt][:, et * 128 : (et + 1) * 128],
                            rhs=pt_sb[kt][:, q * 512 : (q + 1) * 512],
                            start=(kt == 0),
                            stop=(kt == NKT - 1),
                        )
                for q in range(NQC):
                    ot_t = ot_pool.tile([128, 512], F32, tag="ot")
                    nc.vector.tensor_mul(
                        out=ot_t, in0=acc[q], in1=bc_sb[:, q * 512 : (q + 1) * 512]
                    )
                    nc.sync.dma_start(
                        out=out_ext[
                            et * 128 : (et + 1) * 128, q * 512 : (q + 1) * 512
                        ],
                        in_=ot_t,
                    )
    nc.finalize()
    return nc


def _install_trace_shims():
    """Dev-time only (BASS_TRACE=1): provide the NTFF profile hook this
    image's antenv lacks, and keep trace artifacts local instead of
    uploading them."""
    import sys
    import types

    if "antenv.axon_hooks" not in sys.modules:
        import antenv

        mod = types.ModuleType("antenv.axon_hooks")
        mod._hook = None

        def set_axon_ntff_profile_hook(h):
            mod._hook = h

        def get_axon_ntff_profile_hook():
            return mod._hook

        mod.set_axon_ntff_profile_hook = set_axon_ntff_profile_hook
        mod.get_axon_ntff_profile_hook = get_axon_ntff_profile_hook
        sys.modules["antenv.axon_hooks"] = mod
        antenv.axon_hooks = mod
        try:
            from trn_agent_boot.trn_boot import _ntff_profile_via_ctypes

            hook = _ntff_profile_via_ctypes("/opt/axon/libaxon_pjrt.so")
            if hook is not None:
                set_axon_ntff_profile_hook(hook)
        except Exception:
            pass
    from concourse import bass_utils as bu

    bu.upload_artifacts = lambda tmpdir: tmpdir


def _bf16(a):
    return np.ascontiguousarray(a).astype(ml_dtypes.bfloat16)


def kernel(x, Wq, bq, Wk, bk, Wv, bv):
    global LAST_EXEC_TIME_NS, LAST_TRACE
    x = np.asarray(x, dtype=np.float32)
    wq_h = _bf16(np.asarray(Wq, np.float32))
    wk_h = _bf16(np.asarray(Wk, np.float32))
    wv_h = _bf16(np.asarray(Wv, np.float32))
    bq_h = np.ascontiguousarray(np.asarray(bq, np.float32))
    bk_h = np.ascontiguousarray(np.asarray(bk, np.float32))
    bv_h = _bf16(np.asarray(bv, np.float32))

    in_maps = []
    for core in range(8):
        b, h = divmod(core, 2)
        xt = x[b].T  # [D, S]
        if h:
            xt = np.concatenate([xt[:, SQ:], xt[:, :SQ]], axis=1)
        in_maps.append(
            {
                "xt": _bf16(xt),
                "wq": wq_h,
                "wk": wk_h,
                "wv": wv_h,
                "bq": bq_h,
                "bk": bk_h,
                "bv": bv_h,
            }
        )

    nc = _build()
    kwargs = {}
    if os.environ.get("BASS_TRACE"):
        _install_trace_shims()
        tdir = os.environ.get("BASS_ATTN_TRACE_DIR")
        if tdir:
            os.makedirs(tdir, exist_ok=True)
            kwargs["tmpdir"] = tdir
    res = run_bass_kernel_spmd(nc, in_maps, core_ids=list(range(8)), **kwargs)
    LAST_EXEC_TIME_NS = res.exec_time_ns
    LAST_TRACE = getattr(res, "instructions_and_trace", None)

    out = np.empty((B, S, D), np.float32)
    for core in range(8):
        b, h = divmod(core, 2)
        out[b, h * SQ : (h + 1) * SQ, :] = res.results[core]["out"].T
    return out
